# revision 5
# baseline (speedup 1.0000x reference)
"""Trainium2 Bass kernel for nn_CausalSE: causal cumulative-average pooling
+ squeeze-excite gating, data-parallel over batch (one NeuronCore per batch
element).

Reference math per batch element (D=512, T=8192, chunk=16, Tc=512):
    avg    = cumsum(x, t) / (t+1)
    pooled = avg[:, 15::16]                          # [D, Tc]
    h      = relu(w1 @ pooled + b1)                  # [64, Tc]
    g      = sigmoid(w2 @ h + b2)                    # [D, Tc]
    out    = repeat(g, 16, t)[:, :T] * x

Kernel strategy: w1 @ cumsum(chunksum(x)) == cumsum(chunksum(w1 @ x)), so
the TensorEngine computes y = w1 @ x (64 rows) while x streams in, and the
chunk-reduce/cumsum run on the small [64, Tc] tensor instead of [512, Tc].
x stays resident in SBUF (16 MB) so the final gate-multiply reads it from
SBUF; HBM traffic stays at the 32 MB/core floor (read x + write out).
"""

import sys

for _p in ("/opt/trn_rl_repo",):
    if _p not in sys.path:
        sys.path.insert(0, _p)

import numpy as np

B, D, T = 8, 512, 8192
DH = 64          # bottleneck dim = D // 8
CS = 16          # chunksize
TC = T // CS     # 512 chunks
NCORES = 8
NDT = D // 128   # 4 partition tiles of x / out
TB = 1024        # columns loaded per t-block
NTB = T // TB    # 8 streaming steps
MMN = 512        # matmul moving-operand free-dim limit (fp32)
BLK = 2048       # columns per gate-multiply/store block
NBLK = T // BLK

_compiled_nc = None


def build_nc():
    import concourse.tile as tile
    from concourse import bacc, mybir

    f32 = mybir.dt.float32
    AF = mybir.ActivationFunctionType
    ALU = mybir.AluOpType
    AX = mybir.AxisListType

    # Bacc (not plain Bass): its finalize() runs the TRN2 sync-wait
    # legalization (move_matmul_waits_to_ldweights / event-semaphore
    # splitting) that walrus codegen requires.
    nc = bacc.Bacc("TRN2", target_bir_lowering=False)
    x_d = nc.declare_dram_parameter("x", [D, T], f32, isOutput=False)
    w1t_d = nc.declare_dram_parameter("w1t", [D, DH], f32, isOutput=False)
    b1_d = nc.declare_dram_parameter("b1", [DH], f32, isOutput=False)
    w2t_d = nc.declare_dram_parameter("w2t", [DH, D], f32, isOutput=False)
    b2_d = nc.declare_dram_parameter("b2", [D], f32, isOutput=False)
    scale_d = nc.declare_dram_parameter("scale", [DH, TC], f32, isOutput=False)
    out_d = nc.declare_dram_parameter("out", [D, T], f32, isOutput=True)

    with tile.TileContext(nc) as tc:
        with (
            tc.tile_pool(name="xres", bufs=1) as xres,
            tc.tile_pool(name="small", bufs=1) as small,
            tc.tile_pool(name="psum_y", bufs=2, space="PSUM") as psum_y,
            tc.tile_pool(name="psum_g", bufs=4, space="PSUM") as psum_g,
        ):
            # x resident in SBUF: 4 tiles of [128, 8192] = 16 MB
            xt = [
                xres.tile([128, T], f32, tag=f"x{di}", name=f"x{di}")
                for di in range(NDT)
            ]
            w1s = small.tile([128, NDT, DH], f32, tag="w1")
            w2s = small.tile([DH, D], f32, tag="w2")
            b1s = small.tile([DH, 1], f32, tag="b1")
            b2s = small.tile([128, NDT], f32, tag="b2")
            scl = small.tile([DH, TC], f32, tag="scl")
            sq = small.tile([DH, TC], f32, tag="sq")    # chunk sums of y
            qs = small.tile([DH, TC], f32, tag="qs")    # prefix sums
            h = small.tile([DH, TC], f32, tag="h")
            gs = [
                small.tile([128, TC], f32, tag=f"g{di}", name=f"g{di}")
                for di in range(NDT)
            ]

            # -- replicated weights / constants --
            for ki in range(NDT):
                nc.sync.dma_start(
                    w1s[:, ki, :], w1t_d[ki * 128:(ki + 1) * 128, :]
                )
                nc.sync.dma_start(
                    b2s[:, ki:ki + 1],
                    b2_d[ki * 128:(ki + 1) * 128].unsqueeze(1),
                )
            nc.sync.dma_start(w2s[:], w2t_d[:])
            nc.sync.dma_start(b1s[:], b1_d[:].unsqueeze(1))
            nc.sync.dma_start(scl[:], scale_d[:])

            # -- phase A: stream x in; y = w1 @ x on PE; chunk-reduce to sq --
            for tb in range(NTB):
                for di in range(NDT):
                    nc.sync.dma_start(
                        xt[di][:, tb * TB:(tb + 1) * TB],
                        x_d[di * 128:(di + 1) * 128, tb * TB:(tb + 1) * TB],
                    )
                for nb in range(TB // MMN):
                    c0 = tb * TB + nb * MMN
                    yps = psum_y.tile([DH, MMN], f32, tag="y")
                    for ki in range(NDT):
                        nc.tensor.matmul(
                            yps[:],
                            w1s[:, ki, :],
                            xt[ki][:, c0:c0 + MMN],
                            start=(ki == 0),
                            stop=(ki == NDT - 1),
                        )
                    ch0 = c0 // CS
                    nc.vector.reduce_sum(
                        sq[:, ch0:ch0 + MMN // CS],
                        yps[:].rearrange("p (c j) -> p c j", j=CS),
                        axis=AX.X,
                    )

            # -- phase B: causal prefix, scale, SE bottleneck --
            nc.vector.tensor_tensor_scan(
                qs[:], sq[:], sq[:], 0.0, op0=ALU.add, op1=ALU.bypass
            )
            nc.vector.tensor_mul(qs[:], qs[:], scl[:])
            nc.scalar.activation(h[:], qs[:], AF.Relu, bias=b1s[:, :1])
            for di in range(NDT):
                gps = psum_g.tile([128, TC], f32, tag="g")
                nc.tensor.matmul(
                    gps[:],
                    w2s[:, di * 128:(di + 1) * 128],
                    h[:],
                    start=True,
                    stop=True,
                )
                nc.scalar.activation(
                    gs[di][:], gps[:], AF.Sigmoid, bias=b2s[:, di:di + 1]
                )

            # -- phase C: out = x * repeat(g, 16) in SBUF, then store --
            for di in range(NDT):
                for bl in range(NBLK):
                    xv = xt[di][:, bl * BLK:(bl + 1) * BLK].rearrange(
                        "p (c j) -> p c j", j=CS
                    )
                    gv = (
                        gs[di][:, bl * (BLK // CS):(bl + 1) * (BLK // CS)]
                        .unsqueeze(2)
                        .broadcast_to([128, BLK // CS, CS])
                    )
                    nc.vector.tensor_tensor(xv, xv, gv, op=ALU.mult)
                    nc.sync.dma_start(
                        out_d[di * 128:(di + 1) * 128, bl * BLK:(bl + 1) * BLK],
                        xt[di][:, bl * BLK:(bl + 1) * BLK],
                    )
    # run_bass_via_pjrt serializes nc.m as-is; Bacc defers register
    # allocation and TRN2 sync-wait legalization to finalize(), so it must
    # run here or walrus rejects the BIR.
    nc.finalize()
    return nc


def _host_inputs(x, w1, b1, w2, b2, chunksize):
    x = np.ascontiguousarray(np.asarray(x, dtype=np.float32))
    w1 = np.asarray(w1, dtype=np.float32)
    b1 = np.ascontiguousarray(np.asarray(b1, dtype=np.float32))
    w2 = np.asarray(w2, dtype=np.float32)
    b2 = np.ascontiguousarray(np.asarray(b2, dtype=np.float32))
    cs = int(chunksize)
    assert cs == CS and x.shape == (B, D, T), (cs, x.shape)
    w1t = np.ascontiguousarray(w1.T)                      # [D, DH]
    w2t = np.ascontiguousarray(w2.T)                      # [DH, D]
    scale = np.broadcast_to(
        1.0 / (CS * np.arange(1, TC + 1, dtype=np.float32)), (DH, TC)
    )
    scale = np.ascontiguousarray(scale)
    shared = dict(w1t=w1t, b1=b1, w2t=w2t, b2=b2, scale=scale)
    return x, shared


def kernel(x, w1, b1, w2, b2, chunksize):
    global _compiled_nc
    from concourse.bass_utils import run_bass_kernel_spmd

    x, shared = _host_inputs(x, w1, b1, w2, b2, chunksize)
    if _compiled_nc is None:
        _compiled_nc = build_nc()
    in_maps = [
        {"x": np.ascontiguousarray(x[i]), **shared} for i in range(NCORES)
    ]
    res = run_bass_kernel_spmd(_compiled_nc, in_maps, list(range(NCORES)))
    out = np.stack([res.results[i]["out"] for i in range(NCORES)], axis=0)
    return out


# revision 8
# speedup vs baseline: 1.0581x; 1.0581x over previous
"""Trainium2 Bass kernel for nn_CausalSE: causal cumulative-average pooling
+ squeeze-excite gating, data-parallel over batch (one NeuronCore per batch
element).

Reference math per batch element (D=512, T=8192, chunk=16, Tc=512):
    avg    = cumsum(x, t) / (t+1)
    pooled = avg[:, 15::16]                          # [D, Tc]
    h      = relu(w1 @ pooled + b1)                  # [64, Tc]
    g      = sigmoid(w2 @ h + b2)                    # [D, Tc]
    out    = repeat(g, 16, t)[:, :T] * x

Kernel strategy: w1 @ cumsum(chunksum(x)) == cumsum(chunksum(w1 @ x)), so
the TensorEngine computes y = w1 @ x (64 rows) while x streams in, and the
chunk-reduce/cumsum run on the small [64, Tc] tensor instead of [512, Tc].
x stays resident in SBUF (16 MB) so the final gate-multiply reads it from
SBUF; HBM traffic stays at the 32 MB/core floor (read x + write out).
"""

import sys

for _p in ("/opt/trn_rl_repo",):
    if _p not in sys.path:
        sys.path.insert(0, _p)

import numpy as np

B, D, T = 8, 512, 8192
DH = 64          # bottleneck dim = D // 8
CS = 16          # chunksize
TC = T // CS     # 512 chunks
NCORES = 8
NDT = D // 128   # 4 partition tiles of x / out
TB = 1024        # columns loaded per t-block
NTB = T // TB    # 8 streaming steps
MMN = 512        # matmul moving-operand free-dim limit (fp32)
BLK = 2048       # columns per gate-multiply/store block
NBLK = T // BLK

_compiled_nc = None


def build_nc():
    import concourse.tile as tile
    from concourse import bacc, mybir

    f32 = mybir.dt.float32
    AF = mybir.ActivationFunctionType
    ALU = mybir.AluOpType
    AX = mybir.AxisListType

    # Bacc (not plain Bass): its finalize() runs the TRN2 sync-wait
    # legalization (move_matmul_waits_to_ldweights / event-semaphore
    # splitting) that walrus codegen requires.
    nc = bacc.Bacc("TRN2", target_bir_lowering=False)
    # x and w1t feed the f32r matmul; declaring them float32r end-to-end
    # (same bits as fp32) satisfies the BIR verifier's rounding rule.
    f32r = mybir.dt.float32r
    x_d = nc.declare_dram_parameter("x", [D, T], f32r, isOutput=False)
    w1t_d = nc.declare_dram_parameter("w1t", [D, DH], f32r, isOutput=False)
    b1_d = nc.declare_dram_parameter("b1", [DH], f32, isOutput=False)
    w2t_d = nc.declare_dram_parameter("w2t", [DH, D], f32, isOutput=False)
    b2_d = nc.declare_dram_parameter("b2", [D], f32, isOutput=False)
    scale_d = nc.declare_dram_parameter("scale", [DH, TC], f32, isOutput=False)
    out_d = nc.declare_dram_parameter("out", [D, T], f32, isOutput=True)

    CB = TB // CS  # chunks per t-block (64)

    with tile.TileContext(nc) as tc:
        with (
            tc.tile_pool(name="xres", bufs=1) as xres,
            tc.tile_pool(name="small", bufs=1) as small,
            tc.tile_pool(name="obuf", bufs=6) as obuf,
            tc.tile_pool(name="psum_y", bufs=4, space="PSUM") as psum_y,
            tc.tile_pool(name="psum_g", bufs=4, space="PSUM") as psum_g,
        ):
            # x resident in SBUF: 4 tiles of [128, 8192] = 16 MB
            xt = [
                xres.tile([128, T], f32r, tag=f"x{di}", name=f"x{di}")
                for di in range(NDT)
            ]
            w1s = small.tile([128, NDT, DH], f32r, tag="w1")
            w2s = small.tile([DH, D], f32, tag="w2")
            b1s = small.tile([DH, 1], f32, tag="b1")
            b2s = small.tile([128, NDT], f32, tag="b2")
            scl = small.tile([DH, TC], f32, tag="scl")
            sq = small.tile([DH, TC], f32, tag="sq")    # chunk sums of y
            qs = small.tile([DH, TC], f32, tag="qs")    # prefix sums
            h = small.tile([DH, TC], f32, tag="h")
            gs = [
                small.tile([128, TC], f32, tag=f"g{di}", name=f"g{di}")
                for di in range(NDT)
            ]

            # -- replicated weights / constants --
            for ki in range(NDT):
                nc.sync.dma_start(
                    w1s[:, ki, :], w1t_d[ki * 128:(ki + 1) * 128, :]
                )
                nc.sync.dma_start(
                    b2s[:, ki:ki + 1],
                    b2_d[ki * 128:(ki + 1) * 128].unsqueeze(1),
                )
            nc.sync.dma_start(w2s[:], w2t_d[:])
            nc.sync.dma_start(b1s[:], b1_d[:].unsqueeze(1))
            nc.sync.dma_start(scl[:], scale_d[:])

            # Causal pipeline: the gate for chunk c needs only x[:, :16(c+1)],
            # so each t-block's gates are computed from a running prefix and
            # its gate-multiply + store overlap the next block's loads.
            for tb in range(NTB):
                t0 = tb * TB
                c0 = tb * CB
                for di in range(NDT):
                    nc.sync.dma_start(
                        xt[di][:, t0:t0 + TB],
                        x_d[di * 128:(di + 1) * 128, t0:t0 + TB],
                    )
                # y = w1 @ x on PE (f32r: 4x faster stream, fp32 accumulate)
                for nb in range(TB // MMN):
                    cc0 = t0 + nb * MMN
                    yps = psum_y.tile([DH, MMN], f32, tag="y")
                    for ki in range(NDT):
                        nc.tensor.matmul(
                            yps[:],
                            w1s[:, ki, :],
                            xt[ki][:, cc0:cc0 + MMN],
                            start=(ki == 0),
                            stop=(ki == NDT - 1),
                        )
                    ch0 = cc0 // CS
                    nc.vector.reduce_sum(
                        sq[:, ch0:ch0 + MMN // CS],
                        yps[:].rearrange("p (c j) -> p c j", j=CS),
                        axis=AX.X,
                    )
                # running causal prefix over this block's chunks
                nc.vector.tensor_tensor_scan(
                    qs[:, c0:c0 + CB],
                    sq[:, c0:c0 + CB],
                    sq[:, c0:c0 + CB],
                    0.0 if tb == 0 else qs[:, c0 - 1:c0],
                    op0=ALU.add,
                    op1=ALU.bypass,
                )
                # SE bottleneck for this block's CB gate columns
                nc.vector.tensor_mul(
                    h[:, c0:c0 + CB], qs[:, c0:c0 + CB], scl[:, c0:c0 + CB]
                )
                nc.scalar.activation(
                    h[:, c0:c0 + CB], h[:, c0:c0 + CB], AF.Relu, bias=b1s[:, :1]
                )
                for di in range(NDT):
                    gps = psum_g.tile([128, CB], f32, tag="g")
                    nc.tensor.matmul(
                        gps[:],
                        w2s[:, di * 128:(di + 1) * 128],
                        h[:, c0:c0 + CB],
                        start=True,
                        stop=True,
                    )
                    nc.scalar.activation(
                        gs[di][:, c0:c0 + CB], gps[:], AF.Sigmoid,
                        bias=b2s[:, di:di + 1],
                    )
                    # gate-multiply into a bounce tile (the verifier
                    # rejects non-f32r writes into the f32r xt allocation),
                    # then store this block
                    ot = obuf.tile([128, TB], f32, tag="ob", name="ob")
                    xv = xt[di][:, t0:t0 + TB].bitcast(f32).rearrange(
                        "p (c j) -> p c j", j=CS
                    )
                    gv = (
                        gs[di][:, c0:c0 + CB]
                        .unsqueeze(2)
                        .broadcast_to([128, CB, CS])
                    )
                    ov = ot[:].rearrange("p (c j) -> p c j", j=CS)
                    nc.vector.tensor_tensor(ov, xv, gv, op=ALU.mult)
                    nc.sync.dma_start(
                        out_d[di * 128:(di + 1) * 128, t0:t0 + TB],
                        ot[:],
                    )
    # run_bass_via_pjrt serializes nc.m as-is; Bacc defers register
    # allocation and TRN2 sync-wait legalization to finalize(), so it must
    # run here or walrus rejects the BIR.
    nc.finalize()
    return nc


def _host_inputs(x, w1, b1, w2, b2, chunksize):
    x = np.ascontiguousarray(np.asarray(x, dtype=np.float32))
    w1 = np.asarray(w1, dtype=np.float32)
    b1 = np.ascontiguousarray(np.asarray(b1, dtype=np.float32))
    w2 = np.asarray(w2, dtype=np.float32)
    b2 = np.ascontiguousarray(np.asarray(b2, dtype=np.float32))
    cs = int(chunksize)
    assert cs == CS and x.shape == (B, D, T), (cs, x.shape)
    w1t = np.ascontiguousarray(w1.T)                      # [D, DH]
    w2t = np.ascontiguousarray(w2.T)                      # [DH, D]
    scale = np.broadcast_to(
        1.0 / (CS * np.arange(1, TC + 1, dtype=np.float32)), (DH, TC)
    )
    scale = np.ascontiguousarray(scale)
    shared = dict(w1t=w1t, b1=b1, w2t=w2t, b2=b2, scale=scale)
    return x, shared


def kernel(x, w1, b1, w2, b2, chunksize):
    global _compiled_nc
    from concourse.bass_utils import run_bass_kernel_spmd

    x, shared = _host_inputs(x, w1, b1, w2, b2, chunksize)
    if _compiled_nc is None:
        _compiled_nc = build_nc()
    in_maps = [
        {"x": np.ascontiguousarray(x[i]), **shared} for i in range(NCORES)
    ]
    res = run_bass_kernel_spmd(_compiled_nc, in_maps, list(range(NCORES)))
    out = np.stack([res.results[i]["out"] for i in range(NCORES)], axis=0)
    return out


# revision 9
# speedup vs baseline: 1.1213x; 1.0597x over previous
"""Trainium2 Bass kernel for nn_CausalSE: causal cumulative-average pooling
+ squeeze-excite gating, data-parallel over batch (one NeuronCore per batch
element).

Reference math per batch element (D=512, T=8192, chunk=16, Tc=512):
    avg    = cumsum(x, t) / (t+1)
    pooled = avg[:, 15::16]                          # [D, Tc]
    h      = relu(w1 @ pooled + b1)                  # [64, Tc]
    g      = sigmoid(w2 @ h + b2)                    # [D, Tc]
    out    = repeat(g, 16, t)[:, :T] * x

Kernel structure (all fp32, causally pipelined over 2048-col t-blocks):
    s = chunk-sums of x on DVE (windowed reduce, [512, Tc])
    q = w1 @ s on PE (small), prefix via native tensor_tensor_scan with a
    carried initial, then the SE bottleneck and the gate-multiply + store —
    the gate for chunk c needs only x[:, :16(c+1)], so each block's store
    overlaps the next block's loads and HBM streams continuously at the
    32 MB/core floor. x stays resident in SBUF; the multiply is in-place.
"""

import sys

for _p in ("/opt/trn_rl_repo",):
    if _p not in sys.path:
        sys.path.insert(0, _p)

import numpy as np

B, D, T = 8, 512, 8192
DH = 64          # bottleneck dim = D // 8
CS = 16          # chunksize
TC = T // CS     # 512 chunks
NCORES = 8
NDT = D // 128   # 4 partition tiles of x / out
TB = 2048        # columns per t-block (8 KB DMA rows)
NTB = T // TB    # 4 pipeline steps
CB = TB // CS    # chunks per t-block (128)

_compiled_nc = None


def build_nc():
    import concourse.tile as tile
    from concourse import bacc, mybir

    f32 = mybir.dt.float32
    AF = mybir.ActivationFunctionType
    ALU = mybir.AluOpType
    AX = mybir.AxisListType

    # Bacc (not plain Bass): its finalize() runs the TRN2 sync-wait
    # legalization (move_matmul_waits_to_ldweights / event-semaphore
    # splitting) that walrus codegen requires.
    nc = bacc.Bacc("TRN2", target_bir_lowering=False)
    x_d = nc.declare_dram_parameter("x", [D, T], f32, isOutput=False)
    w1t_d = nc.declare_dram_parameter("w1t", [D, DH], f32, isOutput=False)
    b1_d = nc.declare_dram_parameter("b1", [DH], f32, isOutput=False)
    w2t_d = nc.declare_dram_parameter("w2t", [DH, D], f32, isOutput=False)
    b2_d = nc.declare_dram_parameter("b2", [D], f32, isOutput=False)
    scale_d = nc.declare_dram_parameter("scale", [DH, TC], f32, isOutput=False)
    out_d = nc.declare_dram_parameter("out", [D, T], f32, isOutput=True)

    with tile.TileContext(nc) as tc:
        with (
            tc.tile_pool(name="xres", bufs=1) as xres,
            tc.tile_pool(name="small", bufs=1) as small,
            tc.tile_pool(name="psum_q", bufs=4, space="PSUM") as psum_q,
            tc.tile_pool(name="psum_g", bufs=4, space="PSUM") as psum_g,
        ):
            # x resident in SBUF: 4 tiles of [128, 8192] = 16 MB
            xt = [
                xres.tile([128, T], f32, tag=f"x{di}", name=f"x{di}")
                for di in range(NDT)
            ]
            st = [
                small.tile([128, TC], f32, tag=f"s{di}", name=f"s{di}")
                for di in range(NDT)
            ]
            w1s = small.tile([128, NDT, DH], f32, tag="w1")
            w2s = small.tile([DH, D], f32, tag="w2")
            b1s = small.tile([DH, 1], f32, tag="b1")
            b2s = small.tile([128, NDT], f32, tag="b2")
            scl = small.tile([DH, TC], f32, tag="scl")
            qs = small.tile([DH, TC], f32, tag="qs")    # causal prefix
            h = small.tile([DH, TC], f32, tag="h")
            gs = [
                small.tile([128, TC], f32, tag=f"g{di}", name=f"g{di}")
                for di in range(NDT)
            ]

            # -- replicated weights / constants --
            for ki in range(NDT):
                nc.sync.dma_start(
                    w1s[:, ki, :], w1t_d[ki * 128:(ki + 1) * 128, :]
                )
                nc.sync.dma_start(
                    b2s[:, ki:ki + 1],
                    b2_d[ki * 128:(ki + 1) * 128].unsqueeze(1),
                )
            nc.sync.dma_start(w2s[:], w2t_d[:])
            nc.sync.dma_start(b1s[:], b1_d[:].unsqueeze(1))
            nc.sync.dma_start(scl[:], scale_d[:])

            # Causal pipeline: gate for chunk c needs only x[:, :16(c+1)].
            for tb in range(NTB):
                t0 = tb * TB
                c0 = tb * CB
                for di in range(NDT):
                    nc.sync.dma_start(
                        xt[di][:, t0:t0 + TB],
                        x_d[di * 128:(di + 1) * 128, t0:t0 + TB],
                    )
                    # chunk sums of x for this block (windowed reduce)
                    nc.vector.reduce_sum(
                        st[di][:, c0:c0 + CB],
                        xt[di][:, t0:t0 + TB].rearrange(
                            "p (c j) -> p c j", j=CS
                        ),
                        axis=AX.X,
                    )
                # q = w1 @ s for this block's chunk columns
                qp = psum_q.tile([DH, CB], f32, tag="q", name="qp")
                for ki in range(NDT):
                    nc.tensor.matmul(
                        qp[:],
                        w1s[:, ki, :],
                        st[ki][:, c0:c0 + CB],
                        start=(ki == 0),
                        stop=(ki == NDT - 1),
                    )
                # running causal prefix over this block (carry = last col)
                nc.vector.tensor_tensor_scan(
                    qs[:, c0:c0 + CB],
                    qp[:],
                    scl[:, c0:c0 + CB],
                    0.0 if tb == 0 else qs[:, c0 - 1:c0],
                    op0=ALU.add,
                    op1=ALU.bypass,
                )
                # SE bottleneck for this block's gate columns
                nc.vector.tensor_mul(
                    h[:, c0:c0 + CB], qs[:, c0:c0 + CB], scl[:, c0:c0 + CB]
                )
                nc.scalar.activation(
                    h[:, c0:c0 + CB], h[:, c0:c0 + CB], AF.Relu, bias=b1s[:, :1]
                )
                for di in range(NDT):
                    gp = psum_g.tile([128, CB], f32, tag="g", name="gp")
                    nc.tensor.matmul(
                        gp[:],
                        w2s[:, di * 128:(di + 1) * 128],
                        h[:, c0:c0 + CB],
                        start=True,
                        stop=True,
                    )
                    nc.scalar.activation(
                        gs[di][:, c0:c0 + CB], gp[:], AF.Sigmoid,
                        bias=b2s[:, di:di + 1],
                    )
                    # gate-multiply in place in SBUF, then store this block
                    xv = xt[di][:, t0:t0 + TB].rearrange(
                        "p (c j) -> p c j", j=CS
                    )
                    gv = (
                        gs[di][:, c0:c0 + CB]
                        .unsqueeze(2)
                        .broadcast_to([128, CB, CS])
                    )
                    nc.vector.tensor_tensor(xv, xv, gv, op=ALU.mult)
                    nc.sync.dma_start(
                        out_d[di * 128:(di + 1) * 128, t0:t0 + TB],
                        xt[di][:, t0:t0 + TB],
                    )
    # run_bass_via_pjrt serializes nc.m as-is; Bacc defers register
    # allocation and TRN2 sync-wait legalization to finalize(), so it must
    # run here or walrus rejects the BIR.
    nc.finalize()
    return nc


def _host_inputs(x, w1, b1, w2, b2, chunksize):
    x = np.ascontiguousarray(np.asarray(x, dtype=np.float32))
    w1 = np.asarray(w1, dtype=np.float32)
    b1 = np.ascontiguousarray(np.asarray(b1, dtype=np.float32))
    w2 = np.asarray(w2, dtype=np.float32)
    b2 = np.ascontiguousarray(np.asarray(b2, dtype=np.float32))
    cs = int(chunksize)
    assert cs == CS and x.shape == (B, D, T), (cs, x.shape)
    w1t = np.ascontiguousarray(w1.T)                      # [D, DH]
    w2t = np.ascontiguousarray(w2.T)                      # [DH, D]
    scale = np.broadcast_to(
        1.0 / (CS * np.arange(1, TC + 1, dtype=np.float32)), (DH, TC)
    )
    scale = np.ascontiguousarray(scale)
    shared = dict(w1t=w1t, b1=b1, w2t=w2t, b2=b2, scale=scale)
    return x, shared


def kernel(x, w1, b1, w2, b2, chunksize):
    global _compiled_nc
    from concourse.bass_utils import run_bass_kernel_spmd

    x, shared = _host_inputs(x, w1, b1, w2, b2, chunksize)
    if _compiled_nc is None:
        _compiled_nc = build_nc()
    in_maps = [
        {"x": np.ascontiguousarray(x[i]), **shared} for i in range(NCORES)
    ]
    res = run_bass_kernel_spmd(_compiled_nc, in_maps, list(range(NCORES)))
    out = np.stack([res.results[i]["out"] for i in range(NCORES)], axis=0)
    return out


# revision 11
# speedup vs baseline: 1.2536x; 1.1180x over previous
"""Trainium2 Bass kernel for nn_CausalSE: causal cumulative-average pooling
+ squeeze-excite gating, data-parallel over batch (one NeuronCore per batch
element).

Reference math per batch element (D=512, T=8192, chunk=16, Tc=512):
    avg    = cumsum(x, t) / (t+1)
    pooled = avg[:, 15::16]                          # [D, Tc]
    h      = relu(w1 @ pooled + b1)                  # [64, Tc]
    g      = sigmoid(w2 @ h + b2)                    # [D, Tc]
    out    = repeat(g, 16, t)[:, :T] * x

Kernel structure (all fp32, causally pipelined over 2048-col t-blocks):
    s = chunk-sums of x on DVE (windowed reduce, [512, Tc])
    q = w1 @ s on PE (small), prefix via native tensor_tensor_scan with a
    carried initial, then the SE bottleneck and the gate-multiply + store —
    the gate for chunk c needs only x[:, :16(c+1)], so each block's store
    overlaps the next block's loads and HBM streams continuously at the
    32 MB/core floor. x stays resident in SBUF; the multiply is in-place.
"""

import sys

for _p in ("/opt/trn_rl_repo",):
    if _p not in sys.path:
        sys.path.insert(0, _p)

import numpy as np

B, D, T = 8, 512, 8192
DH = 64          # bottleneck dim = D // 8
CS = 16          # chunksize
TC = T // CS     # 512 chunks
NCORES = 8
NDT = D // 128   # 4 partition tiles of x / out
TB = 2048        # columns per t-block (8 KB DMA rows)
NTB = T // TB    # 4 pipeline steps
CB = TB // CS    # chunks per t-block (128)

_compiled_nc = None


def build_nc():
    import concourse.tile as tile
    from concourse import bacc, mybir

    f32 = mybir.dt.float32
    AF = mybir.ActivationFunctionType
    ALU = mybir.AluOpType
    AX = mybir.AxisListType

    # Bacc (not plain Bass): its finalize() runs the TRN2 sync-wait
    # legalization (move_matmul_waits_to_ldweights / event-semaphore
    # splitting) that walrus codegen requires.
    nc = bacc.Bacc("TRN2", target_bir_lowering=False)
    x_d = nc.declare_dram_parameter("x", [D, T], f32, isOutput=False)
    w1t_d = nc.declare_dram_parameter("w1t", [D, DH], f32, isOutput=False)
    b1_d = nc.declare_dram_parameter("b1", [DH], f32, isOutput=False)
    w2t_d = nc.declare_dram_parameter("w2t", [DH, D], f32, isOutput=False)
    b2_d = nc.declare_dram_parameter("b2", [D], f32, isOutput=False)
    scale_d = nc.declare_dram_parameter("scale", [DH, TC], f32, isOutput=False)
    out_d = nc.declare_dram_parameter("out", [D, T], f32, isOutput=True)

    with tile.TileContext(nc) as tc:
        with (
            tc.tile_pool(name="xres", bufs=1) as xres,
            tc.tile_pool(name="small", bufs=1) as small,
            tc.tile_pool(name="psum_q", bufs=4, space="PSUM") as psum_q,
            tc.tile_pool(name="psum_g", bufs=4, space="PSUM") as psum_g,
        ):
            # x resident in SBUF: 4 tiles of [128, 8192] = 16 MB
            xt = [
                xres.tile([128, T], f32, tag=f"x{di}", name=f"x{di}")
                for di in range(NDT)
            ]
            st = [
                small.tile([128, TC], f32, tag=f"s{di}", name=f"s{di}")
                for di in range(NDT)
            ]
            w1s = small.tile([128, NDT, DH], f32, tag="w1")
            w2s = small.tile([DH, D], f32, tag="w2")
            b1s = small.tile([DH, 1], f32, tag="b1")
            b2s = small.tile([128, NDT], f32, tag="b2")
            scl = small.tile([DH, TC], f32, tag="scl")
            qs = small.tile([DH, TC], f32, tag="qs")    # causal prefix
            h = small.tile([DH, TC], f32, tag="h")
            gs = [
                small.tile([128, TC], f32, tag=f"g{di}", name=f"g{di}")
                for di in range(NDT)
            ]

            # -- replicated weights / constants --
            for ki in range(NDT):
                nc.sync.dma_start(
                    w1s[:, ki, :], w1t_d[ki * 128:(ki + 1) * 128, :]
                )
                nc.sync.dma_start(
                    b2s[:, ki:ki + 1],
                    b2_d[ki * 128:(ki + 1) * 128].unsqueeze(1),
                )
            nc.sync.dma_start(w2s[:], w2t_d[:])
            nc.sync.dma_start(b1s[:], b1_d[:].unsqueeze(1))
            nc.sync.dma_start(scl[:], scale_d[:])

            # All loads issue up front on the sync queue: nothing may sit
            # between them, or an in-order store wait would stall prefetch.
            for tb in range(NTB):
                t0 = tb * TB
                for di in range(NDT):
                    nc.sync.dma_start(
                        xt[di][:, t0:t0 + TB],
                        x_d[di * 128:(di + 1) * 128, t0:t0 + TB],
                    )

            # Causal pipeline: gate for chunk c needs only x[:, :16(c+1)].
            for tb in range(NTB):
                t0 = tb * TB
                c0 = tb * CB
                for di in range(NDT):
                    # chunk sums of x for this block (windowed reduce)
                    nc.vector.reduce_sum(
                        st[di][:, c0:c0 + CB],
                        xt[di][:, t0:t0 + TB].rearrange(
                            "p (c j) -> p c j", j=CS
                        ),
                        axis=AX.X,
                    )
                # q = w1 @ s for this block's chunk columns
                qp = psum_q.tile([DH, CB], f32, tag="q", name="qp")
                for ki in range(NDT):
                    nc.tensor.matmul(
                        qp[:],
                        w1s[:, ki, :],
                        st[ki][:, c0:c0 + CB],
                        start=(ki == 0),
                        stop=(ki == NDT - 1),
                    )
                # running causal prefix over this block (carry = last col)
                nc.vector.tensor_tensor_scan(
                    qs[:, c0:c0 + CB],
                    qp[:],
                    scl[:, c0:c0 + CB],
                    0.0 if tb == 0 else qs[:, c0 - 1:c0],
                    op0=ALU.add,
                    op1=ALU.bypass,
                )
                # SE bottleneck for this block's gate columns
                nc.vector.tensor_mul(
                    h[:, c0:c0 + CB], qs[:, c0:c0 + CB], scl[:, c0:c0 + CB]
                )
                nc.scalar.activation(
                    h[:, c0:c0 + CB], h[:, c0:c0 + CB], AF.Relu, bias=b1s[:, :1]
                )
                for di in range(NDT):
                    gp = psum_g.tile([128, CB], f32, tag="g", name="gp")
                    nc.tensor.matmul(
                        gp[:],
                        w2s[:, di * 128:(di + 1) * 128],
                        h[:, c0:c0 + CB],
                        start=True,
                        stop=True,
                    )
                    nc.scalar.activation(
                        gs[di][:, c0:c0 + CB], gp[:], AF.Sigmoid,
                        bias=b2s[:, di:di + 1],
                    )
                    # gate-multiply in place in SBUF (DVE for d0/d1,
                    # idle GpSimd for d2/d3), then store from the same
                    # engine so no wait blocks another queue
                    xv = xt[di][:, t0:t0 + TB].rearrange(
                        "p (c j) -> p c j", j=CS
                    )
                    gv = (
                        gs[di][:, c0:c0 + CB]
                        .unsqueeze(2)
                        .broadcast_to([128, CB, CS])
                    )
                    if di < 2:
                        nc.vector.tensor_tensor(xv, xv, gv, op=ALU.mult)
                        eng = nc.scalar   # ACT may issue DMAs; DVE may not
                    else:
                        nc.gpsimd.tensor_tensor(xv, xv, gv, op=ALU.mult)
                        eng = nc.gpsimd
                    eng.dma_start(
                        out_d[di * 128:(di + 1) * 128, t0:t0 + TB],
                        xt[di][:, t0:t0 + TB],
                    )
    # run_bass_via_pjrt serializes nc.m as-is; Bacc defers register
    # allocation and TRN2 sync-wait legalization to finalize(), so it must
    # run here or walrus rejects the BIR.
    nc.finalize()
    return nc


def _host_inputs(x, w1, b1, w2, b2, chunksize):
    x = np.ascontiguousarray(np.asarray(x, dtype=np.float32))
    w1 = np.asarray(w1, dtype=np.float32)
    b1 = np.ascontiguousarray(np.asarray(b1, dtype=np.float32))
    w2 = np.asarray(w2, dtype=np.float32)
    b2 = np.ascontiguousarray(np.asarray(b2, dtype=np.float32))
    cs = int(chunksize)
    assert cs == CS and x.shape == (B, D, T), (cs, x.shape)
    w1t = np.ascontiguousarray(w1.T)                      # [D, DH]
    w2t = np.ascontiguousarray(w2.T)                      # [DH, D]
    scale = np.broadcast_to(
        1.0 / (CS * np.arange(1, TC + 1, dtype=np.float32)), (DH, TC)
    )
    scale = np.ascontiguousarray(scale)
    shared = dict(w1t=w1t, b1=b1, w2t=w2t, b2=b2, scale=scale)
    return x, shared


def kernel(x, w1, b1, w2, b2, chunksize):
    global _compiled_nc
    from concourse.bass_utils import run_bass_kernel_spmd

    x, shared = _host_inputs(x, w1, b1, w2, b2, chunksize)
    if _compiled_nc is None:
        _compiled_nc = build_nc()
    in_maps = [
        {"x": np.ascontiguousarray(x[i]), **shared} for i in range(NCORES)
    ]
    res = run_bass_kernel_spmd(_compiled_nc, in_maps, list(range(NCORES)))
    out = np.stack([res.results[i]["out"] for i in range(NCORES)], axis=0)
    return out
